# revision 36
# baseline (speedup 1.0000x reference)
"""Multi-head attention (B=2, N=2048, C=1024, H=16, D=64) on 8 Trainium2
NeuronCores.

Sharding: tensor-parallel over heads x data-parallel over batch.
Core (b, g) with b in {0,1}, g in {0..3} handles batch b and heads
[4g, 4g+4). Each core computes qkv for its heads, attention, and a partial
output projection (row-parallel); the host sums the 4 partials per batch and
adds the bias.

v2: bf16 datapath (x, weights, q/k, v, attn, ao) with f32 PSUM accumulation.
bf16 weights trigger the compiler's fast-weight-load (FWL) path, halving the
LDWEIGHTS serialization that dominated the f32r version's PE overhead, and
halve the input DMA (6 MB vs 12 MB) so attention starts ~12us earlier.

Per-core kernel:
  qT/kT [d, n] via lhsT=w^T, rhs=x^T            (d on partitions, pair-packed)
  scoresT[j, i] = kT.T @ qT                     (two K=64 matmuls, row-tiled
                                                 concurrently via base_partition)
  attnT = exp(scale * scoresT)                  (ACT, PSUM->SBUF, no max pass)
  aoT[d, i] += [v | 1]^T @ attnT                (row 64 = softmax denominators)
  aoT = av * (1/sums) fused cast PSUM->bf16 SBUF, then out = aoT.T @ wpT.

The ACT engine needs 147us (128 exp instrs at (1024+352)/1.2GHz); the PE
needs ~166us (stream + residual LDW). Emission order packs all non-attention
PE work (qkv chains, projection) into the ACT-bound attention iterations.
"""
import numpy as np
import os
import sys

sys.path.insert(0, "/opt/trn_rl_repo")

B = 2
N = 2048
C = 1024
H = 16
D = 64
SCALE = D ** -0.5

HEADS_PER_CORE = 4  # 2 pairs
N_CORES = 8

_cache = {}


def _build():
    import concourse.bass as bass
    import concourse.tile as tile
    from concourse import bacc, mybir

    F32 = mybir.dt.float32
    BF16 = mybir.dt.bfloat16
    P = 128
    NC4 = N // 512   # 4 i-chunks of 512
    NB = N // P      # 16 n/j blocks of 128
    CO = C // P      # 8 contraction subtiles

    n_warm = int(os.environ.get("K_WARMUP", "14"))

    nc = bacc.Bacc("TRN2", target_bir_lowering=False, debug=False)
    xT = nc.dram_tensor("xT", (C, N), BF16, kind="ExternalInput")
    wqkT = nc.dram_tensor("wqkT", (C, 512), BF16, kind="ExternalInput")
    wvT = nc.dram_tensor("wvT", (C, 256), BF16, kind="ExternalInput")
    wpT = nc.dram_tensor("wpT", (256, C), BF16, kind="ExternalInput")
    # Per-pair partial projections; the host sums them (it already sums the
    # 4 head-group partials, so 8 arrays instead of 4 is free). This lets
    # pair-0's projection run mid-kernel instead of serializing the tail
    # behind the last attention chunk.
    out0 = nc.dram_tensor("out0", (N, C), BF16, kind="ExternalOutput")
    out1 = nc.dram_tensor("out1", (N, C), BF16, kind="ExternalOutput")
    outs = (out0, out1)

    with tile.TileContext(nc) as tc:
        with (
            tc.tile_pool(name="big", bufs=1) as big,
            tc.tile_pool(name="attn", bufs=6) as attn_pool,
            tc.tile_pool(name="norm", bufs=2) as norm_pool,
            tc.tile_pool(name="outp", bufs=3) as out_pool,
            tc.tile_pool(name="ps_mm", bufs=2, space="PSUM") as ps_mm,
            tc.tile_pool(name="ps_sc", bufs=2, space="PSUM") as ps_sc,
            tc.tile_pool(name="ps_av", bufs=1, space="PSUM") as ps_av,
        ):
            # ---- input DMAs, ordered by first use:
            # x cols 0:512 -> wqk (k0/q0 chains) -> wv (v 0-3) -> x 512:1024
            # (v 4-7, kT1, q1) -> x 1024:2048 (rest) -> wp (proj, last).
            xT_sb = big.tile([P, CO, N], BF16)
            for co in range(CO):
                nc.sync.dma_start(
                    xT_sb[:, co, 0:512], xT.ap()[co * P:(co + 1) * P, 0:512]
                )
            wqk_sb = big.tile([P, CO, 512], BF16)
            for co in range(CO):
                nc.sync.dma_start(
                    wqk_sb[:, co, :], wqkT.ap()[co * P:(co + 1) * P, :]
                )
            wv_sb = big.tile([P, CO, 256], BF16)
            for co in range(CO):
                nc.sync.dma_start(
                    wv_sb[:, co, :], wvT.ap()[co * P:(co + 1) * P, :]
                )
            for co in range(CO):
                nc.sync.dma_start(
                    xT_sb[:, co, 512:1024],
                    xT.ap()[co * P:(co + 1) * P, 512:1024],
                )
            for co in range(CO):
                nc.sync.dma_start(
                    xT_sb[:, co, 1024:2048],
                    xT.ap()[co * P:(co + 1) * P, 1024:2048],
                )
            wp_sb = big.tile([P, 2, C], BF16)
            for cs in range(2):
                nc.sync.dma_start(
                    wp_sb[:, cs, :], wpT.ap()[cs * P:(cs + 1) * P, :]
                )

            # Warmup immediately: HAM un-throttles after ~3.4us of PE
            # activity; run zero matmuls during the DMA lead-in.
            warm = big.tile([P, 512], BF16)
            nc.vector.memset(warm[:], 0.0)
            wsink = big.tile([P, 8], F32)
            for wu in range(n_warm):
                pw = ps_mm.tile([P, 512], F32, name="pwarm", tag="pm")
                nc.tensor.matmul(
                    pw[:], warm[:, 0:128], warm[:], start=True, stop=True
                )
                if wu == n_warm - 1:
                    nc.vector.tensor_copy(wsink[:], pw[:, 0:8])

            ones_c = big.tile([P, 1], BF16)
            nc.vector.memset(ones_c[:], 1.0)
            # Preload the exp ACT table (~2.7us) during the DMA lead-in.
            exp_warm = big.tile([P, 1], F32)
            nc.scalar.activation(
                out=exp_warm[:], in_=ones_c[:],
                func=mybir.ActivationFunctionType.Exp,
            )

            qk_sb = [big.tile([P, N], BF16, name=f"qk_sb{i}") for i in range(4)]
            # per-j-block v tiles (fine-grained deps so attention j=0 does not
            # wait for the whole v phase)
            v_ones = [
                big.tile([P, HEADS_PER_CORE, 65], BF16, name=f"vo{nb}")
                for nb in range(NB)
            ]
            aoT_sb = [big.tile([P, N], BF16, name=f"aoT_sb{i}") for i in range(2)]

            _qk_pending = {}

            def qk_chain_half(fc, ick, half):
                """Half of a qT/kT chain (co 0-3 or 4-7+copy)."""
                if half == 0:
                    pm = ps_mm.tile([P, 512], F32, name="pm", tag="pm")
                    _qk_pending[(fc, ick)] = pm
                else:
                    pm = _qk_pending.pop((fc, ick))
                for co in range(4 * half, 4 * half + 4):
                    nc.tensor.matmul(
                        pm[:],
                        wqk_sb[:, co, fc * P:(fc + 1) * P],
                        xT_sb[:, co, ick * 512:(ick + 1) * 512],
                        start=(co == 0),
                        stop=(co == CO - 1),
                    )
                if half == 1:
                    nc.vector.tensor_copy(
                        qk_sb[fc][:, ick * 512:(ick + 1) * 512], pm[:]
                    )

            def qk_chain(fc, ick):
                qk_chain_half(fc, ick, 0)
                qk_chain_half(fc, ick, 1)

            def v_chain(nb):
                pm = ps_mm.tile([P, 512], F32, name="pm", tag="pm")
                for co in range(CO):
                    nc.tensor.matmul(
                        pm[:, 0:256],
                        xT_sb[:, co, nb * P:(nb + 1) * P],
                        wv_sb[:, co, :],
                        start=(co == 0),
                        stop=(co == CO - 1),
                    )
                nc.vector.tensor_copy(
                    v_ones[nb][:, :, 0:64],
                    pm[:, 0:256].rearrange("p (h d) -> p h d", h=HEADS_PER_CORE),
                )
                nc.vector.tensor_copy(
                    v_ones[nb][:, :, 64:65],
                    ones_c.unsqueeze(1).to_broadcast((P, HEADS_PER_CORE, 1)),
                )

            def norm_slice(pair, av_A, av_B, isl_base, i0, w, tail=False):
                """Normalize av[:, i0:i0+w] (PSUM) into aoT bf16 columns
                [isl_base+i0, isl_base+i0+w). v1-style chain: copy sums out,
                reciprocal, gpsimd broadcast from partition 0, dup to the
                upper half, then multiply in place. With tail=True the bulk
                PSUM->bf16 casts run on the (by then idle) Scalar engine so
                the DVE chain shortens."""
                osl = slice(isl_base + i0, isl_base + i0 + w)
                sl = slice(i0, i0 + w)
                cp = (
                    (lambda o, i: nc.scalar.activation(
                        out=o, in_=i,
                        func=mybir.ActivationFunctionType.Copy))
                    if tail else nc.vector.tensor_copy
                )
                # Copy everything out of the av PSUM banks FIRST: the next
                # chunk's AV accumulation reuses them (ps_av bufs=1), so the
                # whole PE pipeline stalls until these copies retire.
                sumsA = norm_pool.tile([1, 512], F32, name="sumsA")
                sumsB = norm_pool.tile([1, 512], F32, name="sumsB")
                nc.vector.tensor_copy(sumsA[:, sl], av_A[64:65, sl])
                cp(aoT_sb[pair][0:64, osl], av_A[0:64, sl])
                nc.vector.tensor_copy(sumsB[:, sl], av_B[64:65, sl])
                cp(aoT_sb[pair][64:128, osl], av_B[0:64, sl])
                recA = norm_pool.tile([1, 512], F32, name="recA")
                recB = norm_pool.tile([1, 512], F32, name="recB")
                nc.vector.reciprocal_approx_fast(
                    out=recA[:, sl], in_=sumsA[:, sl]
                )
                nc.vector.reciprocal_approx_fast(
                    out=recB[:, sl], in_=sumsB[:, sl]
                )
                rbcA = norm_pool.tile([64, 512], F32, name="rbcA")
                rbcB = norm_pool.tile([P, 512], F32, name="rbcB")
                nc.gpsimd.partition_broadcast(rbcA[:, sl], recA[:, sl])
                nc.gpsimd.partition_broadcast(rbcB[0:64, sl], recB[:, sl])
                nc.vector.tensor_copy(rbcB[64:128, sl], rbcB[0:64, sl])
                nc.vector.tensor_mul(
                    aoT_sb[pair][0:64, osl], aoT_sb[pair][0:64, osl],
                    rbcA[:, sl],
                )
                nc.vector.tensor_mul(
                    aoT_sb[pair][64:128, osl], aoT_sb[pair][64:128, osl],
                    rbcB[64:128, sl],
                )

            def attention_chunk(pair, ick, filler=None, n_norm_slices=1):
                q_t = qk_sb[2 * pair]
                k_t = qk_sb[2 * pair + 1]
                hA = 2 * pair
                hB = 2 * pair + 1
                isl = slice(ick * 512, (ick + 1) * 512)
                av_A = ps_av.tile([65, 512], F32, name="av_A")
                av_B = ps_av.tile([65, 512], F32, name="av_B")

                at_tiles = {}

                def scores_exp(jb):
                    jsl = slice(jb * P, (jb + 1) * P)
                    sc = ps_sc.tile([P, 2, 512], F32, name="sc")
                    nc.tensor.matmul(
                        sc[:, 0, :], k_t[0:64, jsl], q_t[0:64, isl],
                        start=True, stop=True,
                    )
                    nc.tensor.matmul(
                        sc[:, 1, :], k_t[64:128, jsl], q_t[64:128, isl],
                        start=True, stop=True,
                    )
                    at = attn_pool.tile([P, 2, 512], BF16, name="at")
                    nc.scalar.activation(
                        out=at[:], in_=sc[:],
                        func=mybir.ActivationFunctionType.Exp,
                        scale=float(SCALE),
                    )
                    at_tiles[jb] = at

                # Software-pipelined emission: scores/exp run 2 iterations
                # ahead of AV so the PE's in-order queue never parks on
                # AV(jb) waiting for exp(jb) while scores(jb+1) is ready.
                scores_exp(0)
                scores_exp(1)
                for jb in range(NB):
                    at = at_tiles.pop(jb)
                    nc.tensor.matmul(
                        av_A[:], v_ones[jb][:, hA, :], at[:, 0, :],
                        start=(jb == 0), stop=(jb == NB - 1),
                    )
                    nc.tensor.matmul(
                        av_B[:], v_ones[jb][:, hB, :], at[:, 1, :],
                        start=(jb == 0), stop=(jb == NB - 1),
                    )
                    if jb + 2 < NB:
                        scores_exp(jb + 2)
                    # low-priority filler (emitted after the latency-critical
                    # attention ops of this iteration)
                    if filler is not None and jb in filler:
                        filler[jb]()
                # Normalization: aoT = av * (1/sums), PSUM f32 -> bf16.
                w = 512 // n_norm_slices
                for s in range(n_norm_slices):
                    norm_slice(pair, av_A, av_B, ick * 512, s * w, w,
                               tail=(n_norm_slices > 1))

            def proj_half(pair, nb, fck, tail=False):
                nsl = slice(nb * P, (nb + 1) * P)
                fsl = slice(fck * 512, (fck + 1) * 512)
                pj = ps_mm.tile([P, 512], F32, name="pj", tag="pm")
                nc.tensor.matmul(
                    pj[:], aoT_sb[pair][:, nsl], wp_sb[:, pair, fsl],
                    start=True, stop=True,
                )
                ot = out_pool.tile([P, 512], BF16, name="ot")
                if tail and fck == 0:
                    # Alternate the tail PSUM->SBUF copies between the (idle)
                    # Scalar engine and the DVE so neither serializes.
                    nc.scalar.activation(
                        out=ot[:], in_=pj[:],
                        func=mybir.ActivationFunctionType.Copy,
                    )
                else:
                    nc.vector.tensor_copy(ot[:], pj[:])
                nc.sync.dma_start(outs[pair].ap()[nsl, fsl], ot[:])

            # ---- emission: attention p0 starts after the minimal deps
            # (k chunk 0, q chunk 0, v blocks 0-1); everything else — v
            # chains, rest of k_p0/q_p0, pair-1 qkv, the projection —
            # interleaves into the attention iterations as lower-priority
            # PE filler.
            qk_chain(1, 0)        # k_p0 cols 0:512  (j-blocks 0-3)
            qk_chain(0, 0)        # q_p0 cols 0:512  (i-chunk 0)
            v_chain(0)
            v_chain(1)

            def multi(*fns):
                def run():
                    for f in fns:
                        f()
                return run

            # chunk (0,0): v blocks + k_p0 chunks just-in-time (v_nb needed
            # at jb=nb, kT chunk c needed at jb=4c), q_p0 c1 late.
            # Filler at slot jb is emitted after AV(jb) and scores(jb+2), so:
            # v_chain(nb) must sit at a slot strictly below nb, and kT chunk
            # c (qk_chain(1, c)) must fully precede scores(4c), i.e. sit at
            # slots <= 4c - 3. q_p0 chunk 1 goes early to un-gate chunk 1's
            # scores stream.
            f0 = {jb: [] for jb in range(NB)}
            f0[0].append(lambda: qk_chain_half(1, 1, 0))
            f0[0].append(lambda: qk_chain_half(1, 1, 1))
            f0[1].append(lambda: v_chain(2))
            f0[1].append(lambda: v_chain(3))
            f0[2].append(lambda: v_chain(4))
            f0[2].append(lambda: v_chain(5))
            f0[3].append(lambda: qk_chain_half(0, 1, 0))
            f0[4].append(lambda: qk_chain_half(0, 1, 1))
            f0[4].append(lambda: qk_chain_half(1, 2, 0))
            f0[5].append(lambda: qk_chain_half(1, 2, 1))
            f0[5].append(lambda: v_chain(6))
            f0[6].append(lambda: v_chain(7))
            f0[6].append(lambda: v_chain(8))
            f0[7].append(lambda: v_chain(9))
            f0[8].append(lambda: qk_chain_half(1, 3, 0))
            f0[8].append(lambda: v_chain(10))
            f0[9].append(lambda: qk_chain_half(1, 3, 1))
            f0[9].append(lambda: v_chain(11))
            f0[10].append(lambda: v_chain(12))
            f0[11].append(lambda: v_chain(13))
            f0[12].append(lambda: v_chain(14))
            f0[13].append(lambda: v_chain(15))
            attention_chunk(
                0, 0, filler={jb: multi(*fns) for jb, fns in f0.items() if fns}
            )
            def add_fill(fill, slot, fn):
                prev = fill.get(slot)
                fill[slot] = multi(prev, fn) if prev else fn

            def proj_fill(fill, pair, ick, slots):
                base = 4 * ick
                for q in range(8):
                    add_fill(
                        fill, slots[q],
                        lambda nb=base + q // 2, f=q % 2, p=pair:
                        proj_half(p, nb, f),
                    )

            # p0 chunks 1-3: q_p0 rest, pair-1 k/q half-chains, and pair-0's
            # projection chunks as they become ready.
            f1 = {2: lambda: qk_chain_half(0, 2, 0),
                  5: lambda: qk_chain_half(0, 2, 1),
                  8: lambda: qk_chain_half(0, 3, 0),
                  11: lambda: qk_chain_half(0, 3, 1)}
            proj_fill(f1, 0, 0, [3, 4, 6, 7, 9, 10, 12, 13])
            attention_chunk(0, 1, filler=f1)
            attention_chunk(
                0, 2,
                filler={q + 2: (lambda q=q: qk_chain_half(3, q // 2, q % 2))
                        for q in range(8)},
            )
            f3 = {q + 2: (lambda q=q: qk_chain_half(2, q // 2, q % 2))
                  for q in range(4)}
            proj_fill(f3, 0, 1, [6, 7, 8, 9, 10, 11, 12, 13])
            attention_chunk(0, 3, filler=f3)
            # pair 1; pair-1's proj for chunk c interleaves into chunk c+1,
            # pair-0's remaining proj chunks fill the early pair-1 chunks.
            for ick in range(NC4):
                fill = {}
                if ick == 0:
                    fill = {q + 2: (lambda q=q:
                                    qk_chain_half(2, 2 + q // 2, q % 2))
                            for q in range(4)}
                    proj_fill(fill, 0, 2, [6, 7, 8, 9, 10, 11, 12, 13])
                if ick == 1:
                    proj_fill(fill, 0, 3, [6, 7, 8, 9, 10, 11, 12, 13])
                if ick > 0:
                    proj_fill(fill, 1, ick - 1, [2, 3, 4, 5, 6, 7, 8, 9])
                attention_chunk(
                    1, ick, filler=fill,
                    n_norm_slices=2 if ick == NC4 - 1 else 1,
                )
            # tail: only pair-1's last proj chunk remains; PSUM->SBUF copies
            # ride the idle Scalar engine.
            for nb in range(12, 16):
                proj_half(1, nb, 0, tail=True)
                proj_half(1, nb, 1, tail=True)

    nc.compile()
    return nc


def _get_nc():
    if "nc" not in _cache:
        _cache["nc"] = _build()
    return _cache["nc"]


def _shard_inputs(x, w_qkv, w_proj):
    """Build per-core input dicts (bf16). Core index = b * 4 + g."""
    import ml_dtypes

    bf16 = ml_dtypes.bfloat16
    in_maps = []
    for b in range(B):
        xTb = np.ascontiguousarray(x[b].T).astype(bf16)  # [C, N]
        for g in range(4):
            r = g * 256  # head-group row offset within each of q/k/v sections
            wqkT = np.empty((C, 512), np.float32)
            wqkT[:, 0:128] = w_qkv[r:r + 128].T                  # q pair 0
            wqkT[:, 128:256] = w_qkv[C + r:C + r + 128].T        # k pair 0
            wqkT[:, 256:384] = w_qkv[r + 128:r + 256].T          # q pair 1
            wqkT[:, 384:512] = w_qkv[C + r + 128:C + r + 256].T  # k pair 1
            wvT = np.ascontiguousarray(w_qkv[2 * C + r:2 * C + r + 256].T)
            wpT = np.ascontiguousarray(w_proj[:, r:r + 256].T)
            in_maps.append({
                "xT": xTb,
                "wqkT": wqkT.astype(bf16),
                "wvT": wvT.astype(bf16),
                "wpT": wpT.astype(bf16),
            })
    return in_maps


def kernel(x, w_qkv, w_proj, b_proj, _trace=False):
    from concourse.bass_utils import run_bass_kernel_spmd

    x = np.asarray(x, dtype=np.float32)
    w_qkv = np.asarray(w_qkv, dtype=np.float32)
    w_proj = np.asarray(w_proj, dtype=np.float32)
    b_proj = np.asarray(b_proj, dtype=np.float32)

    nc = _get_nc()
    in_maps = _shard_inputs(x, w_qkv, w_proj)
    res = run_bass_kernel_spmd(
        nc, in_maps, core_ids=list(range(N_CORES)), trace=_trace
    )
    out = np.zeros((B, N, C), np.float32)
    for b in range(B):
        for g in range(4):
            r = res.results[b * 4 + g]
            out[b] += r["out0"].astype(np.float32)
            out[b] += r["out1"].astype(np.float32)
    out += b_proj
    if _trace:
        _cache["last_exec_time_ns"] = res.exec_time_ns
        _cache["last_results"] = res
    return out
